# revision 51
# baseline (speedup 1.0000x reference)
"""BERT encoder (12 layers, B=8 T=512 D=768 H=12) on 8 Trainium2 NeuronCores.

Strategy: pure data parallelism - core b runs the full 12-layer stack for
batch element b. No collectives. All matmul operands are bf16 (the PE
streams bf16 moving data at 1 col/cycle @2.4GHz vs 2 cyc/col for fp32r);
PSUM accumulation stays fp32. The residual stream and LayerNorm math stay
in fp32 (separate bf16 copies feed the matmuls) for accuracy headroom.

Softmax: ACT Exp -> unnormalized P (bf16); PV with an augmented
ones-column emits the denominator as row 64; the two heads' denominator
rows are repacked across partitions by DMA so the DVE iterative
reciprocal runs at free-dim 4, then PE outer products broadcast the
inverse denominators and the normalization multiply happens during the
ctx PSUM->SBUF evac. Scores run one pair ahead of PV so the PE always
has ready work while the ACT exp pipeline catches up.

LayerNorm: mean/var via DVE bn_stats/bn_aggr, rstd = exp(-0.5*ln(var+eps))
on ACT (keeps every ACT function in the exp+ln table set: no table
reloads), fused apply forked across DVE (fp32 copy) and ACT (bf16 copy).

The next layer's Q projection depends only on weights and the constant
query_states, so its column blocks are emitted into the two LayerNorm
stall windows (after out_block1 and at the layer boundary) as PE filler.

Host-side folds (exact, negligible FLOPs):
  - attention scale 1/sqrt(dh) folded into Wq and bq (before bf16 cast)
  - V bias folded through Wo1: b1 = bv @ Wo1 + bo1 (rows of softmax sum to 1)
  - weights pre-reshaped to the SBUF lhsT chunk layout, cast to bf16
Zero biases / zero mask / identity LN affine (which is what
reference.setup_inputs() produces) skip their device ops entirely, but the
general paths are implemented and selected when inputs are nonzero.
"""

import numpy as np

L, B, T, D, H, DH = 12, 8, 512, 768, 12, 64
PD = 128
NKC = D // PD  # 6 contraction chunks
NTC = T // PD  # 4 token chunks
NG = 2         # N-groups per 768-wide output (384 each)
GW = D // NG   # 384
EPS = 1e-12
SCALE = 1.0 / np.sqrt(np.float32(DH))


def _split_excess_waits(nc, mybir, bass_rust, max_waits=1):
    """walrus codegen rejects instructions carrying more than a couple of
    sync waits; hoist excess waits onto same-engine NoOps placed before."""
    n = 0
    for f in nc.m.functions:
        for bb in f.blocks:
            new_insts = []
            changed = False
            for inst in bb.instructions:
                si = inst.sync_info
                if si is not None and len(si.on_wait) > max_waits:
                    waits = list(si.on_wait)
                    excess = waits[: len(waits) - max_waits]
                    for i in range(0, len(excess), max_waits):
                        chunk = excess[i : i + max_waits]
                        n += 1
                        nop = mybir.InstNoOp(
                            name=f"I-waitsplit-{n}", ins=[], outs=[]
                        )
                        nop.engine = inst.engine
                        nop.sync_info = bass_rust.SyncInfo(
                            on_wait=chunk, on_update=[]
                        )
                        new_insts.append(nop)
                        changed = True
                    si.on_wait = waits[len(waits) - max_waits :]
                new_insts.append(inst)
            if changed:
                bb.instructions[:] = new_insts
    return n


def build_nc(flags, split_waits=True):
    """Build the per-core Bass module. flags: dict of general-path toggles."""
    import concourse.bass as bass
    import concourse.tile as tile
    from concourse import mybir

    F32 = mybir.dt.float32
    BF16 = mybir.dt.bfloat16
    AF = mybir.ActivationFunctionType
    OP = mybir.AluOpType

    use_mask = flags["use_mask"]
    use_bq = flags["use_bq"]
    use_bk = flags["use_bk"]
    use_b1 = flags["use_b1"]
    use_b2 = flags["use_b2"]
    use_ln1 = flags["use_ln1"]
    use_ln2 = flags["use_ln2"]

    nc = bass.Bass("TRN2", target_bir_lowering=False, debug=False)

    qs_d = nc.dram_tensor("qs", [T, D], BF16, kind="ExternalInput")
    hs_d = nc.dram_tensor("hs", [T, D], F32, kind="ExternalInput")
    w_d = {
        name: nc.dram_tensor(name, [L, PD, NKC * D], BF16, kind="ExternalInput")
        for name in ("wq", "wk", "wv", "wo1", "wo2")
    }
    iden_d = nc.dram_tensor("iden", [PD, PD], BF16, kind="ExternalInput")
    bq_d = nc.dram_tensor("bq", [PD, L * NKC], F32, kind="ExternalInput") if use_bq else None
    bk_d = nc.dram_tensor("bk", [PD, L * NKC], F32, kind="ExternalInput") if use_bk else None
    mask_d = nc.dram_tensor("mask", [PD, NTC], F32, kind="ExternalInput") if use_mask else None
    sel2_d = nc.dram_tensor("sel2", [2, PD], BF16, kind="ExternalInput")
    vones_d = nc.dram_tensor("vones", [PD, H], BF16, kind="ExternalInput")
    b1_d = nc.dram_tensor("b1bc", [L, PD, D], F32, kind="ExternalInput") if use_b1 else None
    b2_d = nc.dram_tensor("b2bc", [L, PD, D], F32, kind="ExternalInput") if use_b2 else None
    ln1w_d = nc.dram_tensor("ln1wbc", [L, PD, D], F32, kind="ExternalInput") if use_ln1 else None
    ln1b_d = nc.dram_tensor("ln1bbc", [L, PD, D], F32, kind="ExternalInput") if use_ln1 else None
    ln2w_d = nc.dram_tensor("ln2wbc", [L, PD, D], F32, kind="ExternalInput") if use_ln2 else None
    ln2b_d = nc.dram_tensor("ln2bbc", [L, PD, D], F32, kind="ExternalInput") if use_ln2 else None
    out_d = nc.dram_tensor("out", [T, D], F32, kind="ExternalOutput")

    with tile.TileContext(nc) as tc:
        import contextlib

        with contextlib.ExitStack() as ctx:
            p_w = ctx.enter_context(tc.tile_pool(name="w", bufs=3))
            p_qt = ctx.enter_context(tc.tile_pool(name="qt", bufs=6))
            p_hid = ctx.enter_context(tc.tile_pool(name="hid", bufs=8))
            p_ht = ctx.enter_context(tc.tile_pool(name="ht", bufs=6))
            p_act = ctx.enter_context(tc.tile_pool(name="act", bufs=18))
            p_ctx = ctx.enter_context(tc.tile_pool(name="ctxp", bufs=7))
            p_v = ctx.enter_context(tc.tile_pool(name="v", bufs=4))
            p_pt = ctx.enter_context(tc.tile_pool(name="pt", bufs=16))
            p_cr = ctx.enter_context(tc.tile_pool(name="cr", bufs=8))
            p_z = ctx.enter_context(tc.tile_pool(name="z", bufs=3))
            p_sm = ctx.enter_context(tc.tile_pool(name="sm", bufs=2))
            p_c1 = ctx.enter_context(tc.tile_pool(name="c1", bufs=1))
            p_bc = ctx.enter_context(tc.tile_pool(name="bc", bufs=2))
            ps_a = ctx.enter_context(tc.tile_pool(name="psA", bufs=2, space="PSUM"))
            ps_b = ctx.enter_context(tc.tile_pool(name="psB", bufs=3, space="PSUM"))
            ps_c = ctx.enter_context(tc.tile_pool(name="psC", bufs=3, space="PSUM"))

            # ---- one-time constants / inputs ----
            iden = p_c1.tile([PD, PD], BF16, tag="iden")
            nc.sync.dma_start(iden[:], iden_d.ap())
            if use_bq:
                bq_t = p_c1.tile([PD, L * NKC], F32, tag="bq")
                nc.sync.dma_start(bq_t[:], bq_d.ap())
            if use_bk:
                bk_t = p_c1.tile([PD, L * NKC], F32, tag="bk")
                nc.sync.dma_start(bk_t[:], bk_d.ap())
            if use_mask:
                mask_t = p_c1.tile([PD, NTC], F32, tag="mask")
                nc.sync.dma_start(mask_t[:], mask_d.ap())
            sel2_t = p_c1.tile([2, PD], BF16, tag="sel2")
            nc.sync.dma_start(sel2_t[:], sel2_d.ap())
            vones_t = p_c1.tile([PD, H], BF16, tag="vones")
            nc.sync.dma_start(vones_t[:], vones_d.ap())

            qs_n = []
            for tc_i in range(NTC):
                t = p_hid.tile([PD, D], BF16, tag="hidq")
                nc.sync.dma_start(t[:], qs_d.ap()[tc_i * PD : (tc_i + 1) * PD, :])
                qs_n.append((None, t))
            h_tiles = []
            for tc_i in range(NTC):
                t = p_hid.tile([PD, D], F32, tag="hid")
                nc.sync.dma_start(t[:], hs_d.ap()[tc_i * PD : (tc_i + 1) * PD, :])
                tb = p_hid.tile([PD, D], BF16, tag="hidb")
                nc.vector.tensor_copy(tb[:], t[:])
                h_tiles.append((t, tb))

            def transpose_norm_to_T(src_tiles, pool, tag):
                """[T, D] (4x[128,768] bf16) -> [D, T] (6x[128,512] bf16).

                Regular matmul src.T @ I: same PE column cost as transpose
                mode for bf16, but with a standard fp32 PSUM output."""
                out = []
                for kc in range(NKC):
                    pt = ps_b.tile([PD, T], F32, tag="pb")
                    for tc_i in range(NTC):
                        nc.tensor.matmul(
                            pt[:, tc_i * PD : (tc_i + 1) * PD],
                            src_tiles[tc_i][1][:, kc * PD : (kc + 1) * PD],
                            iden[:],
                            start=True,
                            stop=True,
                        )
                    dst = pool.tile([PD, T], BF16, tag=tag)
                    nc.vector.tensor_copy(dst[:], pt[:])
                    out.append(dst)
                return out

            qT = transpose_norm_to_T(qs_n, p_qt, "qt")

            # ---- layers ----
            wq_t = p_w.tile([PD, NKC * D], BF16, tag="w")
            nc.sync.dma_start(wq_t[:], w_d["wq"].ap()[0])
            qt_carry = []
            for l in range(L):
                wk_t = p_w.tile([PD, NKC * D], BF16, tag="w")
                nc.sync.dma_start(wk_t[:], w_d["wk"].ap()[l])
                wv_t = p_w.tile([PD, NKC * D], BF16, tag="w")
                nc.sync.dma_start(wv_t[:], w_d["wv"].ap()[l])

                # Q^T, K^T: [D, T], d_out on partitions; evac on ACT
                def proj_T(w_tile, rhs_tiles, bias_t, use_bias, lw=None,
                           mcs=None, outs=None):
                    outs = [] if outs is None else outs
                    for mc in (range(NKC) if mcs is None else mcs):
                        pp = ps_a.tile([PD, T], F32, tag="pa")
                        for kc in range(NKC):
                            nc.tensor.matmul(
                                pp[:],
                                w_tile[:, kc * D + mc * PD : kc * D + (mc + 1) * PD],
                                rhs_tiles[kc][:],
                                start=(kc == 0),
                                stop=(kc == NKC - 1),
                            )
                        dst = p_act.tile([PD, T], BF16, tag="qk")
                        if use_bias:
                            nc.scalar.activation(
                                dst[:], pp[:], AF.Identity,
                                bias=bias_t[:, lw * NKC + mc : lw * NKC + mc + 1],
                                scale=1.0,
                            )
                        else:
                            nc.scalar.copy(dst[:], pp[:])
                        outs.append(dst)
                    return outs

                # QT needs only weights + the constant qT, so its halves
                # are emitted into the two LN stall windows: mc 0-2 after
                # the previous layer's out_block1, mc 3-5 here (while LN2
                # of the previous layer finishes producing h).
                QT = proj_T(wq_t, qT, bq_t if use_bq else None, use_bq,
                            lw=l, mcs=range(3 if l else 0, NKC),
                            outs=qt_carry)
                qt_carry = QT
                hT = transpose_norm_to_T(h_tiles, p_ht, "ht")
                KT = proj_T(wk_t, hT, bk_t if use_bk else None, use_bk, lw=l)

                # V: augmented normal layout [k, 12*65]; head h at cols
                # 65h..65h+63, ones at col 65h+64 (emits the softmax
                # denominator as row 64 of the PV product).
                V = []
                for tc_i in range(NTC):
                    vt = p_v.tile([PD, H * 65], BF16, tag="v")
                    for ng in range(NG):
                        pp = ps_b.tile([PD, GW], F32, tag="pb")
                        for kc in range(NKC):
                            nc.tensor.matmul(
                                pp[:],
                                hT[kc][:, tc_i * PD : (tc_i + 1) * PD],
                                wv_t[:, kc * D + ng * GW : kc * D + (ng + 1) * GW],
                                start=(kc == 0),
                                stop=(kc == NKC - 1),
                            )
                        dst = vt[:, ng * 390 : (ng + 1) * 390].rearrange(
                            "p (h c) -> p h c", c=65
                        )[:, :, 0:64]
                        src_ = pp[:].rearrange("p (h c) -> p h c", c=64)
                        nc.vector.tensor_copy(dst, src_)
                    ones_dst = vt[:].rearrange("p (h c) -> p h c", c=65)[:, :, 64:65]
                    nc.vector.tensor_copy(
                        ones_dst, vones_t[:].rearrange("p (h o) -> p h o", o=1)
                    )
                    V.append(vt)

                wo1_t = p_w.tile([PD, NKC * D], BF16, tag="w")
                nc.sync.dma_start(wo1_t[:], w_d["wo1"].ap()[l])
                wo2_t = p_w.tile([PD, NKC * D], BF16, tag="w")
                nc.sync.dma_start(wo2_t[:], w_d["wo2"].ap()[l])

                ctxT = [
                    p_ctx.tile([PD, T], BF16, tag="ctx", name=f"ctx{i}")
                    for i in range(NKC)
                ]

                def emit_scores(pair):
                    qtile = QT[pair]
                    ktile = KT[pair]
                    pts_p = {}
                    for sub in range(2):
                        off = 64 * sub
                        for kb in range(NTC):
                            sp = ps_a.tile([PD, T], F32, tag="pa",
                                           name=f"sp{pair}_{sub}_{kb}")
                            nc.tensor.matmul(
                                sp[:],
                                ktile[off : off + 64, kb * PD : (kb + 1) * PD],
                                qtile[off : off + 64, :],
                                start=True,
                                stop=True,
                            )
                            pt = p_pt.tile([PD, T], BF16, tag="pts",
                                           name=f"pt{pair}_{sub}_{kb}")
                            if use_mask:
                                nc.scalar.activation(
                                    pt[:], sp[:], AF.Exp,
                                    bias=mask_t[:, kb : kb + 1], scale=1.0,
                                )
                            else:
                                nc.scalar.activation(
                                    pt[:], sp[:], AF.Exp, bias=0.0, scale=1.0,
                                )
                            pts_p[(sub, kb)] = pt
                    return pts_p

                def emit_pv(pair, pts_p):
                    """PV matmuls; immediate PSUM->SBUF evac (frees the
                    bank fast), denominators packed across partitions by
                    DMA so the DVE reciprocal runs at free-dim size 4."""
                    crs = []
                    for sub in range(2):
                        hh = pair * 2 + sub
                        cp = ps_c.tile([65, T], F32, tag="ctxp",
                                       name=f"cp{hh}")
                        for kb in range(NTC):
                            nc.tensor.matmul(
                                cp[:],
                                V[kb][:, 65 * hh : 65 * hh + 65],
                                pts_p[(sub, kb)][:],
                                start=(kb == 0),
                                stop=(kb == NTC - 1),
                            )
                        cr = p_cr.tile([65, T], F32, tag="cr",
                                       name=f"cr{hh}")
                        nc.vector.tensor_copy(cr[:], cp[:])
                        crs.append(cr)
                    dpk = p_sm.tile([PD, 8], F32, tag="dpk", bufs=6,
                                    name=f"dp{pair}")
                    nc.sync.dma_start(dpk[:, 0:4], crs[0][64:65, :])
                    nc.sync.dma_start(dpk[:, 4:8], crs[1][64:65, :])
                    rpkf = p_sm.tile([PD, 8], F32, tag="rpkf", bufs=6,
                                     name=f"rpf{pair}")
                    nc.vector.reciprocal(rpkf[:], dpk[:])
                    rpk = p_sm.tile([PD, 8], BF16, tag="rpk", bufs=6,
                                    name=f"rp{pair}")
                    nc.vector.tensor_copy(rpk[:], rpkf[:])
                    denr = p_sm.tile([2, T], BF16, tag="denr", bufs=6,
                                     name=f"dr{pair}")
                    nc.sync.dma_start(denr[0:1, :], rpk[:, 0:4])
                    nc.sync.dma_start(denr[1:2, :], rpk[:, 4:8])
                    return crs, denr

                def emit_norm(pair, crs, denr):
                    pr = ps_b.tile([PD, T], F32, tag="pb",
                                   name=f"pr{pair}")
                    nc.tensor.matmul(
                        pr[:], sel2_t[:], denr[:], start=True, stop=True
                    )
                    for sub in range(2):
                        off = 64 * sub
                        nc.vector.tensor_tensor(
                            ctxT[pair][off : off + 64, :],
                            crs[sub][0:64, :],
                            pr[off : off + 64, :],
                            op=OP.mult,
                        )

                # software pipeline: scores run one pair ahead of PV so
                # the PE always has ready work while ACT exps catch up;
                # the den/normalize tail trails one pair behind PV.
                NP = H // 2
                pts_q = {0: emit_scores(0), 1: emit_scores(1)}
                pend = []
                for p in range(NP):
                    crs_denr = emit_pv(p, pts_q.pop(p))
                    if p + 2 < NP:
                        pts_q[p + 2] = emit_scores(p + 2)
                    if len(pend) >= 2:
                        emit_norm(*pend.pop(0))
                    pend.append((p,) + crs_denr)
                for it in pend:
                    emit_norm(*it)

                # ---- output block: z = x @ W + residual, then LN ----
                def out_block(lhsT_tiles, w_tile, res_tiles, badd_d, use_badd,
                              lnw_d_, lnb_d_, use_ln, out_tag, is_last):
                    outs = []
                    if use_badd:
                        badd_t = p_bc.tile([PD, D], F32, tag="badd")
                        nc.sync.dma_start(badd_t[:], badd_d.ap()[l])
                    if use_ln:
                        lnw_t = p_bc.tile([PD, D], F32, tag="lnw")
                        nc.sync.dma_start(lnw_t[:], lnw_d_.ap()[l])
                        lnb_t = p_bc.tile([PD, D], F32, tag="lnb")
                        nc.sync.dma_start(lnb_t[:], lnb_d_.ap()[l])
                    for tc_i in range(NTC):
                        z = p_z.tile([PD, D], F32, tag="z")
                        st6 = p_sm.tile([PD, 2 * 6], F32, tag="st6")
                        for ng in range(NG):
                            pp = ps_b.tile([PD, GW], F32, tag="pb")
                            for kc in range(NKC):
                                nc.tensor.matmul(
                                    pp[:],
                                    lhsT_tiles[kc][:, tc_i * PD : (tc_i + 1) * PD],
                                    w_tile[:, kc * D + ng * GW : kc * D + (ng + 1) * GW],
                                    start=(kc == 0),
                                    stop=(kc == NKC - 1),
                                )
                            sl = slice(ng * GW, (ng + 1) * GW)
                            nc.vector.scalar_tensor_tensor(
                                z[:, sl], pp[:], 1.0, res_tiles[tc_i][0][:, sl],
                                op0=OP.mult, op1=OP.add,
                            )
                            if use_badd:
                                nc.vector.scalar_tensor_tensor(
                                    z[:, sl], z[:, sl], 1.0, badd_t[:, sl],
                                    op0=OP.mult, op1=OP.add,
                                )
                            # layernorm stats per 384-wide half, emitted
                            # right behind each half's residual add so the
                            # first half hides under the second GEMM
                            nc.vector.bn_stats(
                                st6[:, ng * 6 : (ng + 1) * 6], z[:, sl]
                            )
                        # rstd = exp(-0.5*ln(var+eps)) on ACT so the only
                        # ACT table set needed is exp+ln.
                        mv = p_sm.tile([PD, 2], F32, tag="mv")
                        nc.vector.bn_aggr(mv[:], st6[:])
                        veps = p_sm.tile([PD, 1], F32, tag="veps")
                        nc.vector.tensor_scalar(
                            veps[:], mv[:, 1:2], 1.0, EPS, op0=OP.mult, op1=OP.add
                        )
                        lnv = p_sm.tile([PD, 1], F32, tag="lnv")
                        nc.scalar.activation(
                            lnv[:], veps[:], AF.Ln, bias=0.0, scale=1.0
                        )
                        rstd = p_sm.tile([PD, 1], F32, tag="rstd")
                        nc.scalar.activation(
                            rstd[:], lnv[:], AF.Exp, bias=0.0, scale=-0.5
                        )
                        nm = p_sm.tile([PD, 1], F32, tag="nm")
                        nc.vector.tensor_scalar_mul(nm[:], mv[:, 0:1], -1.0)
                        urneg = p_sm.tile([PD, 1], F32, tag="urneg")
                        nc.vector.tensor_tensor(
                            urneg[:], nm[:], rstd[:], op=OP.mult
                        )
                        o = p_hid.tile([PD, D], F32, tag=out_tag)
                        if use_ln:
                            on = p_z.tile([PD, D], F32, tag="on")
                            nc.vector.tensor_scalar(
                                on[:], z[:], rstd[:], urneg[:], op0=OP.mult, op1=OP.add
                            )
                            nc.vector.tensor_tensor(
                                on[:], on[:], lnw_t[:], op=OP.mult
                            )
                            nc.vector.tensor_tensor(
                                o[:], on[:], lnb_t[:], op=OP.add
                            )
                            ob = None
                            if not is_last:
                                ob = p_hid.tile([PD, D], BF16,
                                                tag=out_tag + "b")
                                nc.vector.tensor_copy(ob[:], o[:])
                        else:
                            # forked apply: DVE writes the fp32 residual
                            # copy while ACT writes the bf16 matmul copy
                            nc.vector.tensor_scalar(
                                o[:], z[:], rstd[:], urneg[:], op0=OP.mult, op1=OP.add
                            )
                            ob = None
                            if not is_last:
                                ob = p_hid.tile([PD, D], BF16,
                                                tag=out_tag + "b")
                                nc.scalar.activation(
                                    ob[:], z[:], AF.Identity,
                                    bias=urneg[:], scale=rstd[:],
                                )
                        if is_last:
                            nc.sync.dma_start(
                                out_d.ap()[tc_i * PD : (tc_i + 1) * PD, :], o[:]
                            )
                        outs.append((o, ob))
                    return outs

                a_tiles = out_block(
                    ctxT, wo1_t, h_tiles, b1_d, use_b1,
                    ln1w_d, ln1b_d, use_ln1, "hid", False,
                )
                if l + 1 < L:
                    wq_t = p_w.tile([PD, NKC * D], BF16, tag="w")
                    nc.sync.dma_start(wq_t[:], w_d["wq"].ap()[l + 1])
                    qt_carry = proj_T(
                        wq_t, qT, bq_t if use_bq else None, use_bq,
                        lw=l + 1, mcs=range(0, 3),
                    )
                aT = transpose_norm_to_T(a_tiles, p_ht, "ht")
                h_tiles = out_block(
                    aT, wo2_t, a_tiles, b2_d, use_b2,
                    ln2w_d, ln2b_d, use_ln2, "hid", l == L - 1,
                )

    if split_waits:
        import bass_rust

        _split_excess_waits(nc, mybir, bass_rust)
    return nc


def prep_inputs(inputs):
    """Host-side folds. Returns (flags, per-core list)."""
    import ml_dtypes

    BF = ml_dtypes.bfloat16
    g = {k: np.asarray(v, dtype=np.float32) for k, v in inputs.items()}

    wq_s = g["Wq"] * SCALE
    bq_s = g["bq"] * SCALE
    b1 = np.einsum("ld,ldo->lo", g["bv"], g["Wo1"]) + g["bo1"]
    b2 = g["bo2"]

    flags = {
        "use_mask": bool(np.any(g["attention_mask"])),
        "use_bq": bool(np.any(bq_s)),
        "use_bk": bool(np.any(g["bk"])),
        "use_b1": bool(np.any(b1)),
        "use_b2": bool(np.any(b2)),
        "use_ln1": bool(np.any(g["ln1_w"] != 1.0) or np.any(g["ln1_b"])),
        "use_ln2": bool(np.any(g["ln2_w"] != 1.0) or np.any(g["ln2_b"])),
    }

    def wfmt(w):
        return np.ascontiguousarray(
            w.reshape(L, NKC, PD, D).transpose(0, 2, 1, 3).reshape(L, PD, NKC * D)
        ).astype(BF)

    def _sel(which):
        s = np.zeros((1, PD), dtype=np.float32)
        if which == 0:
            s[0, :64] = 1.0
        else:
            s[0, 64:] = 1.0
        return s

    def bfmt(b):
        return np.ascontiguousarray(
            b.reshape(L, NKC, PD).transpose(2, 0, 1).reshape(PD, L * NKC)
        )

    shared = {
        "wq": wfmt(wq_s),
        "wk": wfmt(g["Wk"]),
        "wv": wfmt(g["Wv"]),
        "wo1": wfmt(g["Wo1"]),
        "wo2": wfmt(g["Wo2"]),
        "iden": np.eye(PD, dtype=np.float32).astype(BF),
        "sel2": np.concatenate([_sel(0), _sel(1)], axis=0).astype(BF),
        "vones": np.ones((PD, H), dtype=np.float32).astype(BF),
    }
    if flags["use_bq"]:
        shared["bq"] = bfmt(bq_s)
    if flags["use_bk"]:
        shared["bk"] = bfmt(g["bk"])
    if flags["use_b1"]:
        shared["b1bc"] = np.ascontiguousarray(
            np.broadcast_to(b1[:, None, :], (L, PD, D))
        )
    if flags["use_b2"]:
        shared["b2bc"] = np.ascontiguousarray(
            np.broadcast_to(b2[:, None, :], (L, PD, D))
        )
    if flags["use_ln1"]:
        shared["ln1wbc"] = np.ascontiguousarray(
            np.broadcast_to(g["ln1_w"][:, None, :], (L, PD, D))
        )
        shared["ln1bbc"] = np.ascontiguousarray(
            np.broadcast_to(g["ln1_b"][:, None, :], (L, PD, D))
        )
    if flags["use_ln2"]:
        shared["ln2wbc"] = np.ascontiguousarray(
            np.broadcast_to(g["ln2_w"][:, None, :], (L, PD, D))
        )
        shared["ln2bbc"] = np.ascontiguousarray(
            np.broadcast_to(g["ln2_b"][:, None, :], (L, PD, D))
        )

    per_core = []
    for b in range(B):
        m = dict(shared)
        m["qs"] = np.ascontiguousarray(g["query_states"][b]).astype(BF)
        m["hs"] = np.ascontiguousarray(g["hidden_states"][b])
        if flags["use_mask"]:
            m["mask"] = np.ascontiguousarray(
                g["attention_mask"][b].reshape(NTC, PD).T
            )
        per_core.append(m)
    return flags, per_core


TRACE = False
LAST_EXEC_NS = None
LAST_RESULTS = None


def kernel(**inputs):
    global LAST_EXEC_NS, LAST_RESULTS
    from concourse.bass_utils import run_bass_kernel_spmd

    flags, per_core = prep_inputs(inputs)
    nc = build_nc(flags)
    kw = {}
    if TRACE:
        kw = dict(trace=True, tmpdir="/root/problem/trace_out")
        import os

        os.makedirs("/root/problem/trace_out", exist_ok=True)
    res = run_bass_kernel_spmd(nc, per_core, core_ids=list(range(B)), **kw)
    LAST_EXEC_NS = res.exec_time_ns
    LAST_RESULTS = res
    out = np.stack([np.asarray(res.results[b]["out"]) for b in range(B)], axis=0)
    return out.astype(np.float32)
